# revision 22
# baseline (speedup 1.0000x reference)
"""Trainium2 Bass kernel for MemoryAugmentation.

Computes, for x[B,C,H,W] and mem[M,H,W] (M=10, H=W=88):
    score[b,c,m] = sum_hw x[b,c,h,w] * mem[m,h,w]
    P = softmax_m(score)
    value[b,c,h,w] = sum_m P[b,c,m] * mem[m,h,w]

Sharding: data-parallel over batch B across 8 NeuronCores (4 batches each);
mem is replicated. Host-side layout prep per shard: x is passed transposed
(xT[k, bc], k = h*w, bc = b*c flattened) so the contraction dim lands on
SBUF partitions without any on-chip transposes; memT ([k, m]) feeds the
PE's stationary operand for the score matmul.

Per-core pipeline:
  Phase A (DMA-in bound): stream xT in [128, 2048] pair-tiles (1 MB DMAs),
    round f32 -> float32r on DVE/ACT (full-rate fp32 matmul mode needs a
    rounding producer), accumulate scoreT[10, 1024] = sum_t memT_t.T @ xT_t
    in two PSUM banks.
  ACT: exp (softmax max-subtraction skipped; |score| is O(6), safe in f32).
  Phase C (DMA-out bound): per 128-row tile, sumexp via an fp32 matmul with
    a ones vector + DVE reciprocal; value[128, 484-chunks] = expT.T @ mem
    (float32r), normalization folded into the PSUM->SBUF drain as a
    per-partition scalar multiply split across DVE and ACT; DMA out.
"""

import sys

import numpy as np

sys.path.insert(0, "/opt/trn_rl_repo")

import concourse.bass as bass  # noqa: E402
import concourse.mybir as mybir  # noqa: E402
import concourse.tile as tile  # noqa: E402
from concourse import bacc  # noqa: E402
from concourse.bass import ts  # noqa: E402
from concourse.bass_utils import run_bass_kernel_spmd  # noqa: E402

N_CORES = 8
B, C, H, W = 32, 256, 88, 88
M = 10
K = H * W  # 7744
S = (B // N_CORES) * C  # 1024 rows per core

F32 = mybir.dt.float32
F32R = mybir.dt.float32r

KT_FULL = K // 128  # 60 full 128-wide k tiles
K_TAIL = K - KT_FULL * 128  # 64
N_PAIR = KT_FULL // 2  # 30 pair-tiles of [256, S]

VCOL = 484  # mm2 column chunk; 16 * 484 = 7744, fits one PSUM bank
N_VJ = K // VCOL
OUT_COLS = K // 4  # 1936 columns per output DMA (~1 MB)


def build_kernel(s_rows=S, use_f32r=True, n_devices=N_CORES):
    nc = bacc.Bacc(
        "TRN2", target_bir_lowering=False, debug=False, num_devices=n_devices
    )
    xT_d = nc.declare_dram_parameter("xt", [K, s_rows], F32, isOutput=False)
    mem_d = nc.declare_dram_parameter("mem", [M, K], F32, isOutput=False)
    memT_d = nc.declare_dram_parameter("memT", [K, M], F32, isOutput=False)
    out_d = nc.declare_dram_parameter("out", [s_rows, K], F32, isOutput=True)

    mmdt = F32R if use_f32r else F32
    n_sub = s_rows // 128  # output row tiles
    n_mm1 = s_rows // 512  # mm1 matmuls per k-tile (512-wide, one PSUM bank)

    with tile.TileContext(nc) as tc:
        with (
            tc.tile_pool(name="const", bufs=1) as const_pool,
            tc.tile_pool(name="stage", bufs=2) as stage_pool,
            tc.tile_pool(name="xin", bufs=4) as x_pool,
            tc.tile_pool(name="xr", bufs=4) as xr_pool,
            tc.tile_pool(name="expT", bufs=1) as expT_pool,
            tc.tile_pool(name="recip", bufs=8) as recip_pool,
            tc.tile_pool(name="vout", bufs=4) as vout_pool,
            tc.tile_pool(name="score_ps", bufs=1, space="PSUM") as score_pool,
            tc.tile_pool(name="sum_ps", bufs=2, space="PSUM") as sum_pool,
            tc.tile_pool(name="val_ps", bufs=4, space="PSUM") as val_pool,
        ):
            # ones feeds the N=1 sumexp matmul, which must stay plain fp32:
            # the fp32r matmul dst-pattern ISA check rejects 1-element dsts.
            ones = const_pool.tile([M, 1], F32)
            nc.gpsimd.memset(ones[:], 1.0)

            # float32r operands need a rounding *producer instruction* for
            # their memory location (a DMA write taints it even through a
            # bitcast): DMA f32 into staging tiles, DVE-copy into f32r tiles.
            def load_rounded(shape, name, dram_ap, n_pieces=1):
                t = const_pool.tile(shape, mmdt, tag=name)
                if not use_f32r:
                    nc.scalar.dma_start(t[:], dram_ap)
                    return t
                if n_pieces == 1:
                    st = stage_pool.tile(shape, F32, tag="stg_" + name)
                    nc.scalar.dma_start(st[:], dram_ap)
                    nc.vector.tensor_copy(t[:], st[:])
                    return t
                assert len(shape) == 2 and shape[1] % n_pieces == 0
                w = shape[1] // n_pieces
                for i in range(n_pieces):
                    st = stage_pool.tile([shape[0], w], F32, tag="stg_" + name)
                    nc.scalar.dma_start(st[:], dram_ap[:, ts(i, w)])
                    nc.vector.tensor_copy(t[:, ts(i, w)], st[:])
                return t

            # memT gates phase A's first matmul — load it first; mem_sb is
            # only needed in phase C and is emitted after the phase-A loop.
            memT_sb = load_rounded(
                [128, KT_FULL, M],
                "memT_r",
                memT_d[: KT_FULL * 128].rearrange("(t p) m -> p t m", p=128),
            )
            memT_tail = load_rounded(
                [K_TAIL, M], "memT_tail_r", memT_d[KT_FULL * 128 :]
            )

            # ---- Phase A: stream xT, accumulate scoreT[10, s_rows] ----
            score_ps = score_pool.tile([M, s_rows], F32)

            def mm1(lhsT, xr, kw, t, last):
                for g in range(n_mm1):
                    nc.tensor.matmul(
                        score_ps[:, ts(g, 512)],
                        lhsT,
                        xr[:kw, ts(g, 512)],
                        start=(t == 0),
                        stop=last,
                    )

            for pair in range(N_PAIR):
                xt_st = x_pool.tile([128, 2, s_rows], F32, tag="x")
                nc.sync.dma_start(
                    xt_st[:],
                    xT_d[pair * 256 : (pair + 1) * 256].rearrange(
                        "(two p) b -> p two b", p=128
                    ),
                )
                xr = xr_pool.tile([128, 2, s_rows], mmdt, tag="xr")
                if use_f32r:
                    # alternate the rounding copy between DVE and ACT
                    if pair % 2 == 0:
                        nc.vector.tensor_copy(xr[:, 0], xt_st[:, 0])
                        nc.scalar.copy(xr[:, 1], xt_st[:, 1])
                    else:
                        nc.scalar.copy(xr[:, 0], xt_st[:, 0])
                        nc.vector.tensor_copy(xr[:, 1], xt_st[:, 1])
                else:
                    xr = xt_st
                for two in range(2):
                    t = pair * 2 + two
                    mm1(memT_sb[:, t, :], xr[:, two], 128, t, False)

            # tail k-tile (64 rows)
            xt_st = x_pool.tile([K_TAIL, s_rows], F32, tag="xtail")
            nc.sync.dma_start(xt_st[:], xT_d[KT_FULL * 128 :])
            if use_f32r:
                xr = xr_pool.tile([K_TAIL, s_rows], mmdt, tag="xrtail")
                nc.vector.tensor_copy(xr[:], xt_st[:])
            else:
                xr = xt_st
            mm1(memT_tail[:], xr[:], K_TAIL, KT_FULL, True)

            mem_sb = load_rounded([M, K], "mem_r", mem_d[:], n_pieces=8)

            expT = expT_pool.tile([M, s_rows], mmdt)
            nc.scalar.activation(
                expT[:], score_ps[:], mybir.ActivationFunctionType.Exp
            )

            # ---- Phase C: sumexp + reciprocal for ALL row tiles upfront, so
            # the value matmuls run back-to-back (keeps the PE HAM-warm) and
            # the PSUM drains never wait on the normalization chain ----
            recips = []
            for sub in range(n_sub):
                sum_ps = sum_pool.tile([128, 1], F32, tag="sum")
                expT_f32 = (
                    expT[:, ts(sub, 128)].bitcast(F32)
                    if use_f32r
                    else expT[:, ts(sub, 128)]
                )
                nc.tensor.matmul(sum_ps[:], expT_f32, ones[:], start=True, stop=True)
                recip = recip_pool.tile([128, 1], F32, tag="recip")
                nc.vector.reciprocal(recip[:], sum_ps[:])
                recips.append(recip)

            for sub in range(n_sub):
                recip = recips[sub]
                r0 = sub * 128
                for half in range(4):
                    vout = vout_pool.tile([128, OUT_COLS], F32, tag="vout")
                    for jj in range(N_VJ // 4):
                        j = half * (N_VJ // 4) + jj
                        val_ps = val_pool.tile([128, VCOL], F32, tag="val")
                        nc.tensor.matmul(
                            val_ps[:],
                            expT[:, ts(sub, 128)],
                            mem_sb[:, ts(j, VCOL)],
                            start=True,
                            stop=True,
                        )
                        # fold the softmax normalization into the PSUM drain,
                        # split across DVE and ACT
                        if jj % 2 == 0:
                            nc.vector.tensor_scalar_mul(
                                vout[:, ts(jj, VCOL)], val_ps[:], recip[:]
                            )
                        else:
                            nc.scalar.mul(vout[:, ts(jj, VCOL)], val_ps[:], recip[:])
                    nc.sync.dma_start(
                        out_d[r0 : r0 + 128, ts(half, OUT_COLS)], vout[:]
                    )
    nc.compile()
    return nc


def _install_ntff_hook():
    """The RL image's antenv lacks axon_hooks; recreate it and wire the
    ctypes NTFF hook from trn_agent_boot so trace=True works under axon."""
    import types

    try:
        from antenv.axon_hooks import get_axon_ntff_profile_hook  # noqa: F401

        return
    except ImportError:
        pass
    import antenv

    mod = types.ModuleType("antenv.axon_hooks")
    _hook = [None]
    mod.set_axon_ntff_profile_hook = lambda h: _hook.__setitem__(0, h)
    mod.get_axon_ntff_profile_hook = lambda: _hook[0]
    sys.modules["antenv.axon_hooks"] = mod
    antenv.axon_hooks = mod
    try:
        if "/root/.axon_site" not in sys.path:
            sys.path.insert(0, "/root/.axon_site")
        from trn_agent_boot.trn_boot import _ntff_profile_via_ctypes

        mod.set_axon_ntff_profile_hook(
            _ntff_profile_via_ctypes("/opt/axon/libaxon_pjrt.so")
        )
    except Exception as e:  # degrade to no-trace
        print("ntff hook install failed:", e)


_NC_CACHE = {}


def _get_nc():
    key = (S, True)
    if key not in _NC_CACHE:
        _NC_CACHE[key] = build_kernel(s_rows=S, use_f32r=True)
    return _NC_CACHE[key]


def kernel(x, mem, _trace=False):
    x = np.asarray(x, dtype=np.float32)
    mem = np.ascontiguousarray(np.asarray(mem, dtype=np.float32))
    assert x.shape == (B, C, H, W) and mem.shape == (M, H, W)

    # host-side shard + layout: per core, xT[k, bc] (contraction dim on
    # SBUF partitions), plus the tiny replicated mem / memT parameters
    xf = x.reshape(N_CORES, S, K)
    xT = np.ascontiguousarray(xf.transpose(0, 2, 1))
    memf = mem.reshape(M, K)
    memT = np.ascontiguousarray(memf.T)

    if _trace:
        _install_ntff_hook()
    nc = _get_nc()
    in_maps = [{"xt": xT[i], "mem": memf, "memT": memT} for i in range(N_CORES)]
    res = run_bass_kernel_spmd(
        nc, in_maps, core_ids=list(range(N_CORES)), trace=_trace
    )
    out = np.concatenate([res.results[i]["out"] for i in range(N_CORES)], axis=0)
    out = out.reshape(B, C, H, W)
    if _trace:
        kernel.last_exec_time_ns = res.exec_time_ns
        kernel.last_results = res
    return out


# revision 23
# speedup vs baseline: 1.2461x; 1.2461x over previous
"""Trainium2 Bass kernel for MemoryAugmentation.

Computes, for x[B,C,H,W] and mem[M,H,W] (M=10, H=W=88):
    score[b,c,m] = sum_hw x[b,c,h,w] * mem[m,h,w]
    P = softmax_m(score)
    value[b,c,h,w] = sum_m P[b,c,m] * mem[m,h,w]

Sharding: data-parallel over batch B across 8 NeuronCores (4 batches each);
mem is replicated. Host-side layout prep per shard: x is passed transposed
(xT[k, bc], k = h*w, bc = b*c flattened) so the contraction dim lands on
SBUF partitions without any on-chip transposes; memT ([k, m]) feeds the
PE's stationary operand for the score matmul.

Two dtype modes for the matmul operands:
  "f16"  — operands shipped/cast to float16 (10-bit mantissa). Runs the
           PE's normal datapath: 1 cycle/row, HAM-warmable to 2.4 GHz, FWL
           weight loads, and input DMA halves (xT ships as fp16). The
           softmax normalization uses the same fp16-rounded exp weights as
           the value matmul, so that rounding largely cancels.
  "f32r" — float32r (11-bit mantissa) full-rate fp32 mode. More accurate,
           but fp32r matmuls run the transpose-path: the HAM clock gate
           never warms (PE stays at 1.2 GHz) and every matmul carries a
           serialized ~260 ns weight load.

Per-core pipeline:
  Phase A (DMA-in bound): stream xT in ~1 MB tiles, accumulate
    scoreT[10, 1024] = sum_t memT_t.T @ xT_t in two PSUM banks.
  ACT: exp (softmax max-subtraction skipped; |score| is O(6), safe in f32).
  Phase C (DMA-out bound): sumexp for all row tiles upfront (fp32 matmul
    with a ones vector + DVE reciprocal), then value[128, 484-chunks] =
    expT.T @ mem with the normalization folded into the PSUM->SBUF drain as
    a per-partition scalar multiply split across DVE and ACT; DMA out.
"""

import sys

import numpy as np

sys.path.insert(0, "/opt/trn_rl_repo")

import concourse.bass as bass  # noqa: E402
import concourse.mybir as mybir  # noqa: E402
import concourse.tile as tile  # noqa: E402
from concourse import bacc  # noqa: E402
from concourse.bass import ts  # noqa: E402
from concourse.bass_utils import run_bass_kernel_spmd  # noqa: E402

N_CORES = 8
B, C, H, W = 32, 256, 88, 88
M = 10
K = H * W  # 7744
S = (B // N_CORES) * C  # 1024 rows per core

F32 = mybir.dt.float32
F32R = mybir.dt.float32r
F16 = mybir.dt.float16

KT_FULL = K // 128  # 60 full 128-wide k tiles
K_TAIL = K - KT_FULL * 128  # 64

VCOL = 484  # mm2 column chunk; 16 * 484 = 7744, fits one PSUM bank
N_VJ = K // VCOL
OUT_COLS = K // 4  # 1936 columns per output DMA (~1 MB)

MODE = "f16"  # "f16" | "f32r" | "f32"


def build_kernel(s_rows=S, mode=MODE, n_devices=N_CORES):
    nc = bacc.Bacc(
        "TRN2", target_bir_lowering=False, debug=False, num_devices=n_devices
    )
    mmdt = {"f16": F16, "f32r": F32R, "f32": F32}[mode]
    in_dt = F16 if mode == "f16" else F32
    use_f32r = mode == "f32r"

    xT_d = nc.declare_dram_parameter("xt", [K, s_rows], in_dt, isOutput=False)
    mem_d = nc.declare_dram_parameter("mem", [M, K], in_dt, isOutput=False)
    memT_d = nc.declare_dram_parameter("memT", [K, M], in_dt, isOutput=False)
    out_d = nc.declare_dram_parameter("out", [s_rows, K], F32, isOutput=True)

    n_sub = s_rows // 128  # output row tiles
    n_mm1 = s_rows // 512  # mm1 matmuls per k-tile (512-wide, one PSUM bank)
    # k-tiles per input DMA: ~1MB transfers either way
    tiles_per_load = 4 if mode == "f16" else 2
    n_loads = KT_FULL // tiles_per_load

    with tile.TileContext(nc) as tc:
        with (
            tc.tile_pool(name="const", bufs=1) as const_pool,
            tc.tile_pool(name="stage", bufs=2) as stage_pool,
            tc.tile_pool(name="xin", bufs=4) as x_pool,
            tc.tile_pool(name="xr", bufs=4) as xr_pool,
            tc.tile_pool(name="expT", bufs=1) as expT_pool,
            tc.tile_pool(name="recip", bufs=8) as recip_pool,
            tc.tile_pool(name="vout", bufs=4) as vout_pool,
            tc.tile_pool(name="score_ps", bufs=1, space="PSUM") as score_pool,
            tc.tile_pool(name="sum_ps", bufs=2, space="PSUM") as sum_pool,
            tc.tile_pool(name="val_ps", bufs=4, space="PSUM") as val_pool,
        ):
            # ones feeds the N=1 sumexp matmul; fp32r's dst-pattern ISA check
            # rejects 1-element dsts, so it stays in the plain input dtype.
            ones = const_pool.tile([M, 1], in_dt)
            nc.gpsimd.memset(ones[:], 1.0)

            # float32r operands need a rounding *producer instruction* for
            # their memory location (a DMA write taints it even through a
            # bitcast): DMA f32 into staging tiles, DVE-copy into f32r tiles.
            # f16 mode ships fp16 from the host and DMAs straight in.
            def load_const(shape, name, dram_ap, n_pieces=1):
                t = const_pool.tile(shape, mmdt, tag=name)
                if not use_f32r:
                    nc.scalar.dma_start(t[:], dram_ap)
                    return t
                if n_pieces == 1:
                    st = stage_pool.tile(shape, F32, tag="stg_" + name)
                    nc.scalar.dma_start(st[:], dram_ap)
                    nc.vector.tensor_copy(t[:], st[:])
                    return t
                assert len(shape) == 2 and shape[1] % n_pieces == 0
                w = shape[1] // n_pieces
                for i in range(n_pieces):
                    st = stage_pool.tile([shape[0], w], F32, tag="stg_" + name)
                    nc.scalar.dma_start(st[:], dram_ap[:, ts(i, w)])
                    nc.vector.tensor_copy(t[:, ts(i, w)], st[:])
                return t

            # memT gates phase A's first matmul — load it first; mem_sb is
            # only needed in phase C and is emitted after the phase-A loop.
            memT_sb = load_const(
                [128, KT_FULL, M],
                "memT_r",
                memT_d[: KT_FULL * 128].rearrange("(t p) m -> p t m", p=128),
            )
            memT_tail = load_const(
                [K_TAIL, M], "memT_tail_r", memT_d[KT_FULL * 128 :]
            )

            # ---- Phase A: stream xT, accumulate scoreT[10, s_rows] ----
            score_ps = score_pool.tile([M, s_rows], F32)

            def mm1(lhsT, xr, kw, t, last):
                for g in range(n_mm1):
                    nc.tensor.matmul(
                        score_ps[:, ts(g, 512)],
                        lhsT,
                        xr[:kw, ts(g, 512)],
                        start=(t == 0),
                        stop=last,
                    )

            for load in range(n_loads):
                tl = tiles_per_load
                xt_st = x_pool.tile([128, tl, s_rows], in_dt, tag="x")
                nc.sync.dma_start(
                    xt_st[:],
                    xT_d[load * 128 * tl : (load + 1) * 128 * tl].rearrange(
                        "(tl p) b -> p tl b", p=128
                    ),
                )
                if use_f32r:
                    xr = xr_pool.tile([128, tl, s_rows], mmdt, tag="xr")
                    # alternate the rounding copy between DVE and ACT
                    for i in range(tl):
                        if (load * tl + i) % 2 == 0:
                            nc.vector.tensor_copy(xr[:, i], xt_st[:, i])
                        else:
                            nc.scalar.copy(xr[:, i], xt_st[:, i])
                else:
                    xr = xt_st
                for i in range(tl):
                    t = load * tl + i
                    mm1(memT_sb[:, t, :], xr[:, i], 128, t, False)

            # tail k-tile (64 rows)
            xt_st = x_pool.tile([K_TAIL, s_rows], in_dt, tag="xtail")
            nc.sync.dma_start(xt_st[:], xT_d[KT_FULL * 128 :])
            if use_f32r:
                xr = xr_pool.tile([K_TAIL, s_rows], mmdt, tag="xrtail")
                nc.vector.tensor_copy(xr[:], xt_st[:])
            else:
                xr = xt_st
            mm1(memT_tail[:], xr[:], K_TAIL, KT_FULL, True)

            mem_sb = load_const(
                [M, K], "mem_r", mem_d[:], n_pieces=8 if use_f32r else 1
            )

            expT = expT_pool.tile([M, s_rows], mmdt)
            nc.scalar.activation(
                expT[:], score_ps[:], mybir.ActivationFunctionType.Exp
            )

            # ---- Phase C: sumexp + reciprocal for ALL row tiles upfront, so
            # the value matmuls run back-to-back (keeps the PE HAM-warm) and
            # the PSUM drains never wait on the normalization chain ----
            recips = []
            for sub in range(n_sub):
                sum_ps = sum_pool.tile([128, 1], F32, tag="sum")
                expT_mm = (
                    expT[:, ts(sub, 128)].bitcast(F32)
                    if use_f32r
                    else expT[:, ts(sub, 128)]
                )
                nc.tensor.matmul(sum_ps[:], expT_mm, ones[:], start=True, stop=True)
                recip = recip_pool.tile([128, 1], F32, tag="recip")
                nc.vector.reciprocal(recip[:], sum_ps[:])
                recips.append(recip)

            for sub in range(n_sub):
                recip = recips[sub]
                r0 = sub * 128
                for half in range(4):
                    vout = vout_pool.tile([128, OUT_COLS], F32, tag="vout")
                    for jj in range(N_VJ // 4):
                        j = half * (N_VJ // 4) + jj
                        val_ps = val_pool.tile([128, VCOL], F32, tag="val")
                        nc.tensor.matmul(
                            val_ps[:],
                            expT[:, ts(sub, 128)],
                            mem_sb[:, ts(j, VCOL)],
                            start=True,
                            stop=True,
                        )
                        # fold the softmax normalization into the PSUM drain,
                        # split across DVE and ACT
                        if jj % 2 == 0:
                            nc.vector.tensor_scalar_mul(
                                vout[:, ts(jj, VCOL)], val_ps[:], recip[:]
                            )
                        else:
                            nc.scalar.mul(vout[:, ts(jj, VCOL)], val_ps[:], recip[:])
                    nc.sync.dma_start(
                        out_d[r0 : r0 + 128, ts(half, OUT_COLS)], vout[:]
                    )
    nc.compile()
    return nc


def _install_ntff_hook():
    """The RL image's antenv lacks axon_hooks; recreate it and wire the
    ctypes NTFF hook from trn_agent_boot so trace=True works under axon."""
    import types

    try:
        from antenv.axon_hooks import get_axon_ntff_profile_hook  # noqa: F401

        return
    except ImportError:
        pass
    import antenv

    mod = types.ModuleType("antenv.axon_hooks")
    _hook = [None]
    mod.set_axon_ntff_profile_hook = lambda h: _hook.__setitem__(0, h)
    mod.get_axon_ntff_profile_hook = lambda: _hook[0]
    sys.modules["antenv.axon_hooks"] = mod
    antenv.axon_hooks = mod
    try:
        if "/root/.axon_site" not in sys.path:
            sys.path.insert(0, "/root/.axon_site")
        from trn_agent_boot.trn_boot import _ntff_profile_via_ctypes

        mod.set_axon_ntff_profile_hook(
            _ntff_profile_via_ctypes("/opt/axon/libaxon_pjrt.so")
        )
    except Exception as e:  # degrade to no-trace
        print("ntff hook install failed:", e)


_NC_CACHE = {}


def _get_nc(mode=MODE):
    key = (S, mode)
    if key not in _NC_CACHE:
        _NC_CACHE[key] = build_kernel(s_rows=S, mode=mode)
    return _NC_CACHE[key]


def kernel(x, mem, _trace=False, _mode=MODE):
    x = np.asarray(x, dtype=np.float32)
    mem = np.ascontiguousarray(np.asarray(mem, dtype=np.float32))
    assert x.shape == (B, C, H, W) and mem.shape == (M, H, W)

    # host-side shard + layout: per core, xT[k, bc] (contraction dim on
    # SBUF partitions), plus the tiny replicated mem / memT parameters
    xf = x.reshape(N_CORES, S, K)
    xT = np.ascontiguousarray(xf.transpose(0, 2, 1))
    memf = mem.reshape(M, K)
    memT = np.ascontiguousarray(memf.T)
    if _mode == "f16":
        xT = xT.astype(np.float16)
        memf = memf.astype(np.float16)
        memT = memT.astype(np.float16)

    if _trace:
        _install_ntff_hook()
    nc = _get_nc(_mode)
    in_maps = [{"xt": xT[i], "mem": memf, "memT": memT} for i in range(N_CORES)]
    res = run_bass_kernel_spmd(
        nc, in_maps, core_ids=list(range(N_CORES)), trace=_trace
    )
    out = np.concatenate([res.results[i]["out"] for i in range(N_CORES)], axis=0)
    out = out.reshape(B, C, H, W)
    if _trace:
        kernel.last_exec_time_ns = res.exec_time_ns
        kernel.last_results = res
    return out


# revision 24
# speedup vs baseline: 1.3405x; 1.0758x over previous
"""Trainium2 Bass kernel for MemoryAugmentation.

Computes, for x[B,C,H,W] and mem[M,H,W] (M=10, H=W=88):
    score[b,c,m] = sum_hw x[b,c,h,w] * mem[m,h,w]
    P = softmax_m(score)
    value[b,c,h,w] = sum_m P[b,c,m] * mem[m,h,w]

Sharding: data-parallel over batch B across 8 NeuronCores (4 batches each);
mem is replicated. Host-side layout prep per shard: x is passed transposed
(xT[k, bc], k = h*w, bc = b*c flattened) so the contraction dim lands on
SBUF partitions without any on-chip transposes; memT ([k, m]) feeds the
PE's stationary operand for the score matmul.

Two dtype modes for the matmul operands:
  "f16"  — operands shipped/cast to float16 (10-bit mantissa). Runs the
           PE's normal datapath: 1 cycle/row, HAM-warmable to 2.4 GHz, FWL
           weight loads, and input DMA halves (xT ships as fp16). The
           softmax normalization uses the same fp16-rounded exp weights as
           the value matmul, so that rounding largely cancels.
  "f32r" — float32r (11-bit mantissa) full-rate fp32 mode. More accurate,
           but fp32r matmuls run the transpose-path: the HAM clock gate
           never warms (PE stays at 1.2 GHz) and every matmul carries a
           serialized ~260 ns weight load.

Per-core pipeline:
  Phase A (DMA-in bound): stream xT in ~1 MB tiles, accumulate
    scoreT[10, 1024] = sum_t memT_t.T @ xT_t in two PSUM banks.
  ACT: exp (softmax max-subtraction skipped; |score| is O(6), safe in f32).
  Phase C (DMA-out bound): sumexp for all row tiles upfront (fp32 matmul
    with a ones vector + DVE reciprocal), then value[128, 484-chunks] =
    expT.T @ mem with the normalization folded into the PSUM->SBUF drain as
    a per-partition scalar multiply split across DVE and ACT; DMA out.
"""

import sys

import numpy as np

sys.path.insert(0, "/opt/trn_rl_repo")

import concourse.bass as bass  # noqa: E402
import concourse.mybir as mybir  # noqa: E402
import concourse.tile as tile  # noqa: E402
from concourse import bacc  # noqa: E402
from concourse.bass import ts  # noqa: E402
from concourse.bass_utils import run_bass_kernel_spmd  # noqa: E402

N_CORES = 8
B, C, H, W = 32, 256, 88, 88
M = 10
K = H * W  # 7744
S = (B // N_CORES) * C  # 1024 rows per core

F32 = mybir.dt.float32
F32R = mybir.dt.float32r
F16 = mybir.dt.float16

KT_FULL = K // 128  # 60 full 128-wide k tiles
K_TAIL = K - KT_FULL * 128  # 64

VCOL = 484  # mm2 column chunk; 16 * 484 = 7744, fits one PSUM bank
N_VJ = K // VCOL
OUT_COLS = K // 4  # 1936 columns per output DMA (~1 MB)

MODE = "f16"  # "f16" | "f32r" | "f32"


def build_kernel(s_rows=S, mode=MODE, n_devices=N_CORES):
    nc = bacc.Bacc(
        "TRN2", target_bir_lowering=False, debug=False, num_devices=n_devices
    )
    mmdt = {"f16": F16, "f32r": F32R, "f32": F32}[mode]
    in_dt = F16 if mode == "f16" else F32
    use_f32r = mode == "f32r"

    xT_d = nc.declare_dram_parameter("xt", [K, s_rows], in_dt, isOutput=False)
    mem_d = nc.declare_dram_parameter("mem", [M, K], in_dt, isOutput=False)
    memT_d = nc.declare_dram_parameter("memT", [K, M], in_dt, isOutput=False)
    out_d = nc.declare_dram_parameter("out", [s_rows, K], F32, isOutput=True)

    n_sub = s_rows // 128  # output row tiles
    n_mm1 = s_rows // 512  # mm1 matmuls per k-tile (512-wide, one PSUM bank)
    # k-tiles per input DMA: ~1MB transfers either way
    tiles_per_load = 4 if mode == "f16" else 2
    n_loads = KT_FULL // tiles_per_load

    with tile.TileContext(nc) as tc:
        with (
            tc.tile_pool(name="const", bufs=1) as const_pool,
            tc.tile_pool(name="stage", bufs=2) as stage_pool,
            tc.tile_pool(name="xin", bufs=6) as x_pool,
            tc.tile_pool(name="xr", bufs=4) as xr_pool,
            tc.tile_pool(name="expT", bufs=1) as expT_pool,
            tc.tile_pool(name="recip", bufs=8) as recip_pool,
            tc.tile_pool(name="vout", bufs=4) as vout_pool,
            tc.tile_pool(name="score_ps", bufs=1, space="PSUM") as score_pool,
            tc.tile_pool(name="sum_ps", bufs=2, space="PSUM") as sum_pool,
            tc.tile_pool(name="val_ps", bufs=4, space="PSUM") as val_pool,
        ):
            # ones feeds the N=1 sumexp matmul; fp32r's dst-pattern ISA check
            # rejects 1-element dsts, so it stays in the plain input dtype.
            ones = const_pool.tile([M, 1], in_dt)
            nc.gpsimd.memset(ones[:], 1.0)

            # float32r operands need a rounding *producer instruction* for
            # their memory location (a DMA write taints it even through a
            # bitcast): DMA f32 into staging tiles, DVE-copy into f32r tiles.
            # f16 mode ships fp16 from the host and DMAs straight in.
            def load_const(shape, name, dram_ap, n_pieces=1):
                t = const_pool.tile(shape, mmdt, tag=name)
                if not use_f32r:
                    nc.scalar.dma_start(t[:], dram_ap)
                    return t
                if n_pieces == 1:
                    st = stage_pool.tile(shape, F32, tag="stg_" + name)
                    nc.scalar.dma_start(st[:], dram_ap)
                    nc.vector.tensor_copy(t[:], st[:])
                    return t
                assert len(shape) == 2 and shape[1] % n_pieces == 0
                w = shape[1] // n_pieces
                for i in range(n_pieces):
                    st = stage_pool.tile([shape[0], w], F32, tag="stg_" + name)
                    nc.scalar.dma_start(st[:], dram_ap[:, ts(i, w)])
                    nc.vector.tensor_copy(t[:, ts(i, w)], st[:])
                return t

            # memT gates phase A's first matmul — load it first; mem_sb is
            # only needed in phase C and is emitted after the phase-A loop.
            memT_sb = load_const(
                [128, KT_FULL, M],
                "memT_r",
                memT_d[: KT_FULL * 128].rearrange("(t p) m -> p t m", p=128),
            )
            memT_tail = load_const(
                [K_TAIL, M], "memT_tail_r", memT_d[KT_FULL * 128 :]
            )

            # ---- Phase A: stream xT, accumulate scoreT[10, s_rows] ----
            score_ps = score_pool.tile([M, s_rows], F32)

            # issue the tiny tail k-tile load up front so the final
            # accumulation step never waits behind the 1 MB streaming loads
            xt_tail = x_pool.tile([K_TAIL, s_rows], in_dt, tag="xtail")
            nc.sync.dma_start(xt_tail[:], xT_d[KT_FULL * 128 :])

            def mm1(lhsT, xr, kw, t, last):
                for g in range(n_mm1):
                    nc.tensor.matmul(
                        score_ps[:, ts(g, 512)],
                        lhsT,
                        xr[:kw, ts(g, 512)],
                        start=(t == 0),
                        stop=last,
                    )

            for load in range(n_loads):
                tl = tiles_per_load
                xt_st = x_pool.tile([128, tl, s_rows], in_dt, tag="x")
                nc.sync.dma_start(
                    xt_st[:],
                    xT_d[load * 128 * tl : (load + 1) * 128 * tl].rearrange(
                        "(tl p) b -> p tl b", p=128
                    ),
                )
                if use_f32r:
                    xr = xr_pool.tile([128, tl, s_rows], mmdt, tag="xr")
                    # alternate the rounding copy between DVE and ACT
                    for i in range(tl):
                        if (load * tl + i) % 2 == 0:
                            nc.vector.tensor_copy(xr[:, i], xt_st[:, i])
                        else:
                            nc.scalar.copy(xr[:, i], xt_st[:, i])
                else:
                    xr = xt_st
                for i in range(tl):
                    t = load * tl + i
                    mm1(memT_sb[:, t, :], xr[:, i], 128, t, False)

            # tail k-tile (64 rows) — data already loaded up front
            if use_f32r:
                xr = xr_pool.tile([K_TAIL, s_rows], mmdt, tag="xrtail")
                nc.vector.tensor_copy(xr[:], xt_tail[:])
            else:
                xr = xt_tail
            mm1(memT_tail[:], xr[:], K_TAIL, KT_FULL, True)

            mem_sb = load_const(
                [M, K], "mem_r", mem_d[:], n_pieces=8 if use_f32r else 1
            )

            expT = expT_pool.tile([M, s_rows], mmdt)
            for g in range(n_mm1):
                nc.scalar.activation(
                    expT[:, ts(g, 512)],
                    score_ps[:, ts(g, 512)],
                    mybir.ActivationFunctionType.Exp,
                )

            # ---- Phase C: sumexp + reciprocal for ALL row tiles upfront, so
            # the value matmuls run back-to-back (keeps the PE HAM-warm) and
            # the PSUM drains never wait on the normalization chain ----
            recips = []
            for sub in range(n_sub):
                sum_ps = sum_pool.tile([128, 1], F32, tag="sum")
                expT_mm = (
                    expT[:, ts(sub, 128)].bitcast(F32)
                    if use_f32r
                    else expT[:, ts(sub, 128)]
                )
                nc.tensor.matmul(sum_ps[:], expT_mm, ones[:], start=True, stop=True)
                recip = recip_pool.tile([128, 1], F32, tag="recip")
                nc.vector.reciprocal(recip[:], sum_ps[:])
                recips.append(recip)

            for sub in range(n_sub):
                recip = recips[sub]
                r0 = sub * 128
                for half in range(4):
                    vout = vout_pool.tile([128, OUT_COLS], F32, tag="vout")
                    for jj in range(N_VJ // 4):
                        j = half * (N_VJ // 4) + jj
                        val_ps = val_pool.tile([128, VCOL], F32, tag="val")
                        nc.tensor.matmul(
                            val_ps[:],
                            expT[:, ts(sub, 128)],
                            mem_sb[:, ts(j, VCOL)],
                            start=True,
                            stop=True,
                        )
                        # fold the softmax normalization into the PSUM drain,
                        # split across DVE and ACT
                        if jj % 2 == 0:
                            nc.vector.tensor_scalar_mul(
                                vout[:, ts(jj, VCOL)], val_ps[:], recip[:]
                            )
                        else:
                            nc.scalar.mul(vout[:, ts(jj, VCOL)], val_ps[:], recip[:])
                    nc.sync.dma_start(
                        out_d[r0 : r0 + 128, ts(half, OUT_COLS)], vout[:]
                    )
    nc.compile()
    return nc


def _install_ntff_hook():
    """The RL image's antenv lacks axon_hooks; recreate it and wire the
    ctypes NTFF hook from trn_agent_boot so trace=True works under axon."""
    import types

    try:
        from antenv.axon_hooks import get_axon_ntff_profile_hook  # noqa: F401

        return
    except ImportError:
        pass
    import antenv

    mod = types.ModuleType("antenv.axon_hooks")
    _hook = [None]
    mod.set_axon_ntff_profile_hook = lambda h: _hook.__setitem__(0, h)
    mod.get_axon_ntff_profile_hook = lambda: _hook[0]
    sys.modules["antenv.axon_hooks"] = mod
    antenv.axon_hooks = mod
    try:
        if "/root/.axon_site" not in sys.path:
            sys.path.insert(0, "/root/.axon_site")
        from trn_agent_boot.trn_boot import _ntff_profile_via_ctypes

        mod.set_axon_ntff_profile_hook(
            _ntff_profile_via_ctypes("/opt/axon/libaxon_pjrt.so")
        )
    except Exception as e:  # degrade to no-trace
        print("ntff hook install failed:", e)


_NC_CACHE = {}


def _get_nc(mode=MODE):
    key = (S, mode)
    if key not in _NC_CACHE:
        _NC_CACHE[key] = build_kernel(s_rows=S, mode=mode)
    return _NC_CACHE[key]


def kernel(x, mem, _trace=False, _mode=MODE):
    x = np.asarray(x, dtype=np.float32)
    mem = np.ascontiguousarray(np.asarray(mem, dtype=np.float32))
    assert x.shape == (B, C, H, W) and mem.shape == (M, H, W)

    # host-side shard + layout: per core, xT[k, bc] (contraction dim on
    # SBUF partitions), plus the tiny replicated mem / memT parameters
    xf = x.reshape(N_CORES, S, K)
    xT = np.ascontiguousarray(xf.transpose(0, 2, 1))
    memf = mem.reshape(M, K)
    memT = np.ascontiguousarray(memf.T)
    if _mode == "f16":
        xT = xT.astype(np.float16)
        memf = memf.astype(np.float16)
        memT = memT.astype(np.float16)

    if _trace:
        _install_ntff_hook()
    nc = _get_nc(_mode)
    in_maps = [{"xt": xT[i], "mem": memf, "memT": memT} for i in range(N_CORES)]
    res = run_bass_kernel_spmd(
        nc, in_maps, core_ids=list(range(N_CORES)), trace=_trace
    )
    out = np.concatenate([res.results[i]["out"] for i in range(N_CORES)], axis=0)
    out = out.reshape(B, C, H, W)
    if _trace:
        kernel.last_exec_time_ns = res.exec_time_ns
        kernel.last_results = res
    return out
